# revision 1
# baseline (speedup 1.0000x reference)
"""GPTQ int4 quantized linear (CaiQuantLinear) on 8 Trainium2 NeuronCores.

y = x @ dequant(qweight, scales, qzeros) + bias
  x: [8192, 4096] f32, qweight: [256, 4096] int64 (16x 4-bit packed along
  infeatures), scales: [32, 4096] f32, qzeros: [32, 256] int64 (packed along
  outfeatures), g_idx = arange(4096)//128, bias: [4096] f32 -> y: [8192, 4096] f32

Sharding: 4 token-shards x 2 outfeature-shards = 8 cores. Core c handles
tokens [2048*(c//2), +2048) and outfeatures [2048*(c%2), +2048).

Device kernel (per core): the packed weights are shipped as one byte per
4-bit-pair row (row k holds the byte containing nibble k, for every o), so
unpack is a single fused per-partition shift+mask tensor_scalar; dequant is
two tensor_tensor ops against k-replicated scale/zero rows; the matmul
accumulates 32 k-tiles of [128,128]x[128,256] bf16 into PSUM, and the
evacuation adds the bias. All replication/transposition is host-side layout
prep so every DMA is a contiguous load.
"""

import sys

if "/opt/trn_rl_repo" not in sys.path:
    sys.path.insert(0, "/opt/trn_rl_repo")

import numpy as np
import ml_dtypes

import concourse.bass as bass  # noqa: F401  (registers mybir types)
import concourse.mybir as mybir
import concourse.tile as tile
from concourse import bacc
from concourse.bass_utils import run_bass_kernel_spmd

BF16 = mybir.dt.bfloat16
F32 = mybir.dt.float32
U8 = mybir.dt.uint8

N_CORES = 8
NT, NO = 4, 2          # token shards x outfeature shards
TOK, IN_F, OUT_F = 8192, 4096, 4096
T = TOK // NT          # 2048 tokens per core
OS = OUT_F // NO       # 2048 outfeatures per core
P = 128
NB = IN_F // P         # 32 contraction super-tiles
OB = 256               # outfeature block (psum free dim)
NOB = OS // OB         # 8
NTB = T // P           # 16 token blocks

_CACHE = {}


def _build_program():
    nc = bacc.Bacc("TRN2", target_bir_lowering=False, debug=False,
                   num_devices=N_CORES)
    xt_ap = nc.dram_tensor("xt", [NB, P, T], BF16, kind="ExternalInput").ap()
    qb_ap = nc.dram_tensor("qb", [NB, NOB, P, OB], U8, kind="ExternalInput").ap()
    sr_ap = nc.dram_tensor("sr", [NB, NOB, P, OB], BF16, kind="ExternalInput").ap()
    zr_ap = nc.dram_tensor("zr", [NB, NOB, P, OB], BF16, kind="ExternalInput").ap()
    br_ap = nc.dram_tensor("br", [P, OS], F32, kind="ExternalInput").ap()
    sh_ap = nc.dram_tensor("sh", [P, 1], U8, kind="ExternalInput").ap()
    y_ap = nc.dram_tensor("y", [NTB, NOB, P, OB], F32, kind="ExternalOutput").ap()

    with tile.TileContext(nc) as tc:
        with tc.tile_pool(name="resident", bufs=1) as rpool, \
             tc.tile_pool(name="wset", bufs=2) as wpool, \
             tc.tile_pool(name="qstream", bufs=4) as qpool, \
             tc.tile_pool(name="ostream", bufs=4) as opool, \
             tc.tile_pool(name="psum", bufs=4, space="PSUM") as ppool:
            sh_sb = rpool.tile([P, 1], U8)
            nc.sync.dma_start(sh_sb[:], sh_ap[:])
            br_sb = rpool.tile([P, OS], F32)
            nc.sync.dma_start(br_sb[:], br_ap[:])
            xt_sb = rpool.tile([P, NB, T], BF16)
            for b in range(NB):
                nc.sync.dma_start(xt_sb[:, b, :], xt_ap[b])

            for ob in range(NOB):
                wset = wpool.tile([P, NB, OB], BF16, tag="wset")
                for b in range(NB):
                    qt = qpool.tile([P, OB], U8, tag="qt")
                    nc.sync.dma_start(qt[:], qb_ap[b, ob])
                    st = qpool.tile([P, OB], BF16, tag="st")
                    nc.sync.dma_start(st[:], sr_ap[b, ob])
                    zt = qpool.tile([P, OB], BF16, tag="zt")
                    nc.sync.dma_start(zt[:], zr_ap[b, ob])
                    wu = qpool.tile([P, OB], U8, tag="wu")
                    nc.vector.tensor_scalar(
                        out=wu[:], in0=qt[:], scalar1=sh_sb[:], scalar2=15,
                        op0=mybir.AluOpType.logical_shift_right,
                        op1=mybir.AluOpType.bitwise_and)
                    nc.vector.tensor_tensor(
                        wset[:, b, :], wu[:], zt[:], mybir.AluOpType.subtract)
                    nc.vector.tensor_tensor(
                        wset[:, b, :], wset[:, b, :], st[:], mybir.AluOpType.mult)
                for tb in range(NTB):
                    ps = ppool.tile([P, OB], F32, tag="ps")
                    for b in range(NB):
                        nc.tensor.matmul(
                            ps[:], xt_sb[:, b, tb * P:(tb + 1) * P], wset[:, b, :],
                            start=(b == 0), stop=(b == NB - 1))
                    ot = opool.tile([P, OB], F32, tag="ot")
                    nc.vector.tensor_tensor(
                        ot[:], ps[:], br_sb[:, ob * OB:(ob + 1) * OB],
                        mybir.AluOpType.add)
                    nc.sync.dma_start(y_ap[tb, ob], ot[:])

    nc.compile()
    return nc


def _host_prep(x, qweight, scales, qzeros, bias):
    """Per-core input maps: pure layout prep (transpose / byte-split /
    row-replication), no arithmetic on the quantized weights."""
    bf16 = ml_dtypes.bfloat16
    x = np.asarray(x, dtype=np.float32)
    qw = np.asarray(qweight).astype(np.int64, copy=False)
    sc = np.asarray(scales, dtype=np.float32)
    qz = np.asarray(qzeros).astype(np.int64, copy=False)
    bi = np.asarray(bias, dtype=np.float32)

    # zeros: unpack along outfeatures, +1 (pack() stored z-1)
    shifts = (np.arange(16, dtype=np.uint64) * np.uint64(4))
    zz = ((qz.astype(np.uint64)[:, :, None] >> shifts[None, None, :])
          & np.uint64(15)).reshape(qz.shape[0], -1).astype(np.float32) + 1.0

    sh_np = (4 * (np.arange(P, dtype=np.uint8) % 2)).reshape(P, 1)

    # per-token-shard xT (shared by the NO cores in a shard row)
    xt_list = []
    for tc in range(NT):
        xs = x[tc * T:(tc + 1) * T]                      # [T, IN_F]
        xt = np.ascontiguousarray(xs.T).astype(bf16)     # [IN_F, T]
        xt_list.append(xt.reshape(NB, P, T))

    # per-outfeature-shard weight-side tensors (shared by NT cores)
    qb_list, sr_list, zr_list, br_list = [], [], [], []
    for oc in range(NO):
        o0 = oc * OS
        qs = np.ascontiguousarray(qw[:, o0:o0 + OS])     # [256, OS] int64
        qbytes = qs.view(np.uint8).reshape(IN_F // 16, OS, 8)
        qb2 = np.ascontiguousarray(qbytes.transpose(0, 2, 1)).reshape(IN_F // 2, OS)
        qb = np.repeat(qb2, 2, axis=0)                   # [IN_F, OS]; row k
        qb_t = np.ascontiguousarray(
            qb.reshape(NB, P, NOB, OB).transpose(0, 2, 1, 3))
        qb_list.append(qb_t)

        s_bf = sc[:, o0:o0 + OS].astype(bf16).reshape(NB, NOB, OB)
        sr_list.append(np.ascontiguousarray(
            np.broadcast_to(s_bf[:, :, None, :], (NB, NOB, P, OB))))
        z_bf = zz[:, o0:o0 + OS].astype(bf16).reshape(NB, NOB, OB)
        zr_list.append(np.ascontiguousarray(
            np.broadcast_to(z_bf[:, :, None, :], (NB, NOB, P, OB))))
        br_list.append(np.ascontiguousarray(
            np.broadcast_to(bi[o0:o0 + OS], (P, OS))))

    in_maps = []
    for c in range(N_CORES):
        tc, oc = c // NO, c % NO
        in_maps.append({
            "xt": xt_list[tc],
            "qb": qb_list[oc],
            "sr": sr_list[oc],
            "zr": zr_list[oc],
            "br": br_list[oc],
            "sh": sh_np,
        })
    return in_maps


def get_program():
    if "nc" not in _CACHE:
        _CACHE["nc"] = _build_program()
    return _CACHE["nc"]


def kernel(x, qweight, scales, qzeros, g_idx, bias):
    nc = get_program()
    in_maps = _host_prep(x, qweight, scales, qzeros, bias)
    res = run_bass_kernel_spmd(nc, in_maps, core_ids=list(range(N_CORES)))
    y = np.empty((TOK, OUT_F), dtype=np.float32)
    for c in range(N_CORES):
        tc, oc = c // NO, c % NO
        yt = res.results[c]["y"]                         # [NTB, NOB, P, OB]
        y[tc * T:(tc + 1) * T, oc * OS:(oc + 1) * OS] = (
            yt.transpose(0, 2, 1, 3).reshape(T, OS))
    return y


# revision 4
# speedup vs baseline: 1.3420x; 1.3420x over previous
"""GPTQ int4 quantized linear (CaiQuantLinear) on 8 Trainium2 NeuronCores.

y = x @ dequant(qweight, scales, qzeros) + bias
  x: [8192, 4096] f32, qweight: [256, 4096] int64 (16x 4-bit packed along
  infeatures), scales: [32, 4096] f32, qzeros: [32, 256] int64 (packed along
  outfeatures), g_idx = arange(4096)//128, bias: [4096] f32 -> y: [8192, 4096] f32

Sharding: 4 token-shards x 2 outfeature-shards = 8 cores. Core c handles
tokens [2048*(c//2), +2048) and outfeatures [2048*(c%2), +2048).

Device kernel (per core): the packed weights are shipped as one byte per
4-bit-pair row (row k holds the byte containing nibble k, for every o), so
unpack is a single fused per-partition shift+mask tensor_scalar; dequant is
two tensor_tensor ops against k-replicated scale/zero rows; the matmul
accumulates 32 k-tiles of [128,128]x[128,256] bf16 into PSUM, and the
evacuation adds the bias. All replication/transposition is host-side layout
prep so every DMA is a contiguous load.
"""

import sys

if "/opt/trn_rl_repo" not in sys.path:
    sys.path.insert(0, "/opt/trn_rl_repo")

import numpy as np
import ml_dtypes

import concourse.bass as bass  # noqa: F401  (registers mybir types)
import concourse.mybir as mybir
import concourse.tile as tile
from concourse import bacc
from concourse.bass_utils import run_bass_kernel_spmd

BF16 = mybir.dt.bfloat16
F32 = mybir.dt.float32
U8 = mybir.dt.uint8

N_CORES = 8
NT, NO = 4, 2          # token shards x outfeature shards
TOK, IN_F, OUT_F = 8192, 4096, 4096
T = TOK // NT          # 2048 tokens per core
OS = OUT_F // NO       # 2048 outfeatures per core
P = 128
NB = IN_F // P         # 32 contraction super-tiles
OB = 256               # outfeature block (psum free dim)
NOB = OS // OB         # 8
NTB = T // P           # 16 token blocks

_CACHE = {}


CB = 4                  # super-tiles per packed stream DMA
NCH = NB // CB          # 8 chunks
BLK = OB + 4 * OB       # 1280 bytes per b: [q u8 | s bf16 | z bf16]


def _build_program():
    nc = bacc.Bacc("TRN2", target_bir_lowering=False, debug=False,
                   num_devices=N_CORES)
    xt_ap = nc.dram_tensor("xt", [NB, P, T], BF16, kind="ExternalInput").ap()
    pk_ap = nc.dram_tensor("pk", [NCH, NOB, P, CB * BLK], U8,
                           kind="ExternalInput").ap()
    br_ap = nc.dram_tensor("br", [P, OS], F32, kind="ExternalInput").ap()
    sh_ap = nc.dram_tensor("sh", [P, 1], U8, kind="ExternalInput").ap()
    y_ap = nc.dram_tensor("y", [NTB, NOB, P, OB], F32, kind="ExternalOutput").ap()

    with tile.TileContext(nc) as tc:
        with tc.tile_pool(name="resident", bufs=1) as rpool, \
             tc.tile_pool(name="wset", bufs=2) as wpool, \
             tc.tile_pool(name="qstream", bufs=3) as qpool, \
             tc.tile_pool(name="ostream", bufs=4) as opool, \
             tc.tile_pool(name="psum", bufs=4, space="PSUM") as ppool:
            sh_sb = rpool.tile([P, 1], U8)
            nc.sync.dma_start(sh_sb[:], sh_ap[:])
            br_sb = rpool.tile([P, OS], F32)
            nc.sync.dma_start(br_sb[:], br_ap[:])
            xt_sb = rpool.tile([P, NB, T], BF16)
            for b in range(NB):
                nc.scalar.dma_start(xt_sb[:, b, :], xt_ap[b])

            for ob in range(NOB):
                wset = wpool.tile([P, NB, OB], BF16, tag="wset")
                for ch in range(NCH):
                    pk_sb = qpool.tile([P, CB * BLK], U8, tag="pk")
                    nc.sync.dma_start(pk_sb[:], pk_ap[ch, ob])
                    for l in range(CB):
                        b = ch * CB + l
                        base = l * BLK
                        qt = pk_sb[:, base:base + OB]
                        st = pk_sb[:, base + OB:base + 3 * OB].bitcast(BF16)
                        zt = pk_sb[:, base + 3 * OB:base + 5 * OB].bitcast(BF16)
                        wu = qpool.tile([P, OB], U8, tag="wu")
                        nc.vector.tensor_scalar(
                            out=wu[:], in0=qt, scalar1=sh_sb[:], scalar2=15,
                            op0=mybir.AluOpType.logical_shift_right,
                            op1=mybir.AluOpType.bitwise_and)
                        nc.vector.tensor_tensor(
                            wset[:, b, :], wu[:], zt, mybir.AluOpType.subtract)
                        nc.vector.tensor_tensor(
                            wset[:, b, :], wset[:, b, :], st,
                            mybir.AluOpType.mult)
                for tb in range(NTB):
                    ps = ppool.tile([P, OB], F32, tag="ps")
                    for b in range(NB):
                        nc.tensor.matmul(
                            ps[:], xt_sb[:, b, tb * P:(tb + 1) * P], wset[:, b, :],
                            start=(b == 0), stop=(b == NB - 1))
                    ot = opool.tile([P, OB], F32, tag="ot")
                    nc.vector.tensor_tensor(
                        ot[:], ps[:], br_sb[:, ob * OB:(ob + 1) * OB],
                        mybir.AluOpType.add)
                    nc.gpsimd.dma_start(y_ap[tb, ob], ot[:])

    nc.compile()
    return nc


def _host_prep(x, qweight, scales, qzeros, bias):
    """Per-core input maps: pure layout prep (transpose / byte-split /
    row-replication), no arithmetic on the quantized weights."""
    bf16 = ml_dtypes.bfloat16
    x = np.asarray(x, dtype=np.float32)
    qw = np.asarray(qweight).astype(np.int64, copy=False)
    sc = np.asarray(scales, dtype=np.float32)
    qz = np.asarray(qzeros).astype(np.int64, copy=False)
    bi = np.asarray(bias, dtype=np.float32)

    # zeros: unpack along outfeatures, +1 (pack() stored z-1)
    shifts = (np.arange(16, dtype=np.uint64) * np.uint64(4))
    zz = ((qz.astype(np.uint64)[:, :, None] >> shifts[None, None, :])
          & np.uint64(15)).reshape(qz.shape[0], -1).astype(np.float32) + 1.0

    sh_np = (4 * (np.arange(P, dtype=np.uint8) % 2)).reshape(P, 1)

    # per-token-shard xT (shared by the NO cores in a shard row)
    xt_list = []
    for tc in range(NT):
        xs = x[tc * T:(tc + 1) * T]                      # [T, IN_F]
        xt = np.ascontiguousarray(xs.T).astype(bf16)     # [IN_F, T]
        xt_list.append(xt.reshape(NB, P, T))

    # per-outfeature-shard weight-side tensors (shared by NT cores):
    # pack [q u8 | s bf16 | z bf16] per (b, ob) into one stream tensor
    pk_list, br_list = [], []
    for oc in range(NO):
        o0 = oc * OS
        qs = np.ascontiguousarray(qw[:, o0:o0 + OS])     # [256, OS] int64
        qbytes = qs.view(np.uint8).reshape(IN_F // 16, OS, 8)
        qb2 = np.ascontiguousarray(qbytes.transpose(0, 2, 1)).reshape(IN_F // 2, OS)
        qb = np.repeat(qb2, 2, axis=0)                   # [IN_F, OS]; row k
        qb_t = np.ascontiguousarray(
            qb.reshape(NB, P, NOB, OB).transpose(0, 2, 1, 3))

        s_bf = sc[:, o0:o0 + OS].astype(bf16).reshape(NB, NOB, OB)
        sr_t = np.ascontiguousarray(
            np.broadcast_to(s_bf[:, :, None, :], (NB, NOB, P, OB)))
        z_bf = zz[:, o0:o0 + OS].astype(bf16).reshape(NB, NOB, OB)
        zr_t = np.ascontiguousarray(
            np.broadcast_to(z_bf[:, :, None, :], (NB, NOB, P, OB)))

        blk = np.concatenate(
            [qb_t, sr_t.view(np.uint8), zr_t.view(np.uint8)],
            axis=-1)                                     # [NB, NOB, P, BLK]
        pk = np.ascontiguousarray(
            blk.reshape(NCH, CB, NOB, P, BLK)
               .transpose(0, 2, 3, 1, 4)
               .reshape(NCH, NOB, P, CB * BLK))
        pk_list.append(pk)
        br_list.append(np.ascontiguousarray(
            np.broadcast_to(bi[o0:o0 + OS], (P, OS))))

    in_maps = []
    for c in range(N_CORES):
        tc, oc = c // NO, c % NO
        in_maps.append({
            "xt": xt_list[tc],
            "pk": pk_list[oc],
            "br": br_list[oc],
            "sh": sh_np,
        })
    return in_maps


def get_program():
    if "nc" not in _CACHE:
        _CACHE["nc"] = _build_program()
    return _CACHE["nc"]


def kernel(x, qweight, scales, qzeros, g_idx, bias):
    nc = get_program()
    in_maps = _host_prep(x, qweight, scales, qzeros, bias)
    res = run_bass_kernel_spmd(nc, in_maps, core_ids=list(range(N_CORES)))
    y = np.empty((TOK, OUT_F), dtype=np.float32)
    for c in range(N_CORES):
        tc, oc = c // NO, c % NO
        yt = res.results[c]["y"]                         # [NTB, NOB, P, OB]
        y[tc * T:(tc + 1) * T, oc * OS:(oc + 1) * OS] = (
            yt.transpose(0, 2, 1, 3).reshape(T, OS))
    return y


# revision 7
# speedup vs baseline: 1.3502x; 1.0062x over previous
"""GPTQ int4 quantized linear (CaiQuantLinear) on 8 Trainium2 NeuronCores.

y = x @ dequant(qweight, scales, qzeros) + bias
  x: [8192, 4096] f32, qweight: [256, 4096] int64 (16x 4-bit packed along
  infeatures), scales: [32, 4096] f32, qzeros: [32, 256] int64 (packed along
  outfeatures), g_idx = arange(4096)//128, bias: [4096] f32 -> y: [8192, 4096] f32

Sharding: 4 token-shards x 2 outfeature-shards = 8 cores. Core c handles
tokens [2048*(c//2), +2048) and outfeatures [2048*(c%2), +2048).

Device kernel (per core): the packed weights are shipped as one byte per
4-bit-pair row (row k holds the byte containing nibble k, for every o), so
unpack is a single fused per-partition shift+mask tensor_scalar; dequant is
two tensor_tensor ops against k-replicated scale/zero rows; the matmul
accumulates 32 k-tiles of [128,128]x[128,256] bf16 into PSUM, and the
evacuation adds the bias. All replication/transposition is host-side layout
prep so every DMA is a contiguous load.
"""

import sys

if "/opt/trn_rl_repo" not in sys.path:
    sys.path.insert(0, "/opt/trn_rl_repo")

import numpy as np
import ml_dtypes

import concourse.bass as bass  # noqa: F401  (registers mybir types)
import concourse.mybir as mybir
import concourse.tile as tile
from concourse import bacc
from concourse.bass_utils import run_bass_kernel_spmd

BF16 = mybir.dt.bfloat16
F32 = mybir.dt.float32
U8 = mybir.dt.uint8

N_CORES = 8
NT, NO = 4, 2          # token shards x outfeature shards
TOK, IN_F, OUT_F = 8192, 4096, 4096
T = TOK // NT          # 2048 tokens per core
OS = OUT_F // NO       # 2048 outfeatures per core
P = 128
NB = IN_F // P         # 32 contraction super-tiles
OB = 256               # outfeature block (psum free dim)
NOB = OS // OB         # 8
NTB = T // P           # 16 token blocks

_CACHE = {}


CB = 4                  # super-tiles per packed stream DMA
NCH = NB // CB          # 8 chunks
I16 = mybir.dt.int16
BLK = 2 * OB + 4 * OB   # 1536 bytes per b: [q i16 | s bf16 | z bf16]


def _build_program():
    nc = bacc.Bacc("TRN2", target_bir_lowering=False, debug=False,
                   num_devices=N_CORES)
    xt_ap = nc.dram_tensor("xt", [NTB, P, NB, P], BF16, kind="ExternalInput").ap()
    pk_ap = nc.dram_tensor("pk", [NCH, NOB, P, CB * BLK], U8,
                           kind="ExternalInput").ap()
    br_ap = nc.dram_tensor("br", [P, OS], F32, kind="ExternalInput").ap()
    sh_ap = nc.dram_tensor("sh", [P, 1], I16, kind="ExternalInput").ap()
    y_ap = nc.dram_tensor("y", [NTB, NOB, P, OB], F32, kind="ExternalOutput").ap()

    with tile.TileContext(nc) as tc:
        with tc.tile_pool(name="resident", bufs=1) as rpool, \
             tc.tile_pool(name="wset", bufs=2) as wpool, \
             tc.tile_pool(name="qstream", bufs=3) as qpool, \
             tc.tile_pool(name="ostream", bufs=4) as opool, \
             tc.tile_pool(name="psum", bufs=4, space="PSUM") as ppool, \
             tc.tile_pool(name="jpsum", bufs=1, space="PSUM") as jpool:
            sh_sb = rpool.tile([P, 1], I16)
            nc.sync.dma_start(sh_sb[:], sh_ap[:])
            br_sb = rpool.tile([P, OS], F32)
            nc.sync.dma_start(br_sb[:], br_ap[:])
            # zeros rhs for HAM-warmup matmuls during the load phase
            wz = rpool.tile([P, OB], BF16)
            nc.gpsimd.memset(wz[:], 0.0)
            jp = jpool.tile([P, OB], F32)

            xt_sb = rpool.tile([P, NB, T], BF16)
            for tb in range(NTB):
                nc.scalar.dma_start(
                    xt_sb[:, :, tb * P:(tb + 1) * P], xt_ap[tb])
                # junk matmul reading the just-loaded slice: keeps the PE
                # HAM-warm through the load phase (psum jp is never read;
                # real accumulations use start=True so this is harmless)
                nc.tensor.matmul(jp[:], xt_sb[:, 0, tb * P:(tb + 1) * P],
                                 wz[:], start=True, stop=True)

            for ob in range(NOB):
                wset = wpool.tile([P, NB, OB], BF16, tag="wset")
                for ch in range(NCH):
                    pk_sb = qpool.tile([P, CB * BLK], U8, tag="pk")
                    nc.sync.dma_start(pk_sb[:], pk_ap[ch, ob])
                    for l in range(CB):
                        b = ch * CB + l
                        base = l * BLK
                        qt = pk_sb[:, base:base + 2 * OB].bitcast(I16)
                        st = pk_sb[:, base + 2 * OB:base + 4 * OB].bitcast(BF16)
                        zt = pk_sb[:, base + 4 * OB:base + 6 * OB].bitcast(BF16)
                        wu = qpool.tile([P, OB], I16, tag="wu")
                        nc.vector.tensor_scalar(
                            out=wu[:], in0=qt, scalar1=sh_sb[:], scalar2=15,
                            op0=mybir.AluOpType.logical_shift_right,
                            op1=mybir.AluOpType.bitwise_and)
                        nc.vector.tensor_tensor(
                            wset[:, b, :], wu[:], zt, mybir.AluOpType.subtract)
                        nc.vector.tensor_tensor(
                            wset[:, b, :], wset[:, b, :], st,
                            mybir.AluOpType.mult)
                    if ob == 0:
                        # keep PE warm while the first wset dequants
                        nc.tensor.matmul(
                            jp[:], wset[:, ch * CB, :P], wz[:],
                            start=True, stop=True)
                for tb in range(NTB):
                    ps = ppool.tile([P, OB], F32, tag="ps")
                    for b in range(NB):
                        nc.tensor.matmul(
                            ps[:], xt_sb[:, b, tb * P:(tb + 1) * P], wset[:, b, :],
                            start=(b == 0), stop=(b == NB - 1))
                    ot = opool.tile([P, OB], F32, tag="ot")
                    nc.vector.tensor_tensor(
                        ot[:], ps[:], br_sb[:, ob * OB:(ob + 1) * OB],
                        mybir.AluOpType.add)
                    nc.gpsimd.dma_start(y_ap[tb, ob], ot[:])

    nc.compile()
    return nc


def _host_prep(x, qweight, scales, qzeros, bias):
    """Per-core input maps: pure layout prep (transpose / byte-split /
    row-replication), no arithmetic on the quantized weights."""
    bf16 = ml_dtypes.bfloat16
    x = np.asarray(x, dtype=np.float32)
    qw = np.asarray(qweight).astype(np.int64, copy=False)
    sc = np.asarray(scales, dtype=np.float32)
    qz = np.asarray(qzeros).astype(np.int64, copy=False)
    bi = np.asarray(bias, dtype=np.float32)

    # zeros: unpack along outfeatures, +1 (pack() stored z-1)
    shifts = (np.arange(16, dtype=np.uint64) * np.uint64(4))
    zz = ((qz.astype(np.uint64)[:, :, None] >> shifts[None, None, :])
          & np.uint64(15)).reshape(qz.shape[0], -1).astype(np.float32) + 1.0

    sh_np = (4 * (np.arange(P, dtype=np.int16) % 2)).reshape(P, 1)

    # per-token-shard xT (shared by the NO cores in a shard row),
    # laid out per token-block so the first matmul group's lhsT arrives fast:
    # [NTB, P(k-part), NB, P(t)]
    xt_list = []
    for tc in range(NT):
        xs = x[tc * T:(tc + 1) * T]                      # [T, IN_F]
        xt = np.ascontiguousarray(xs.T).astype(bf16)     # [IN_F, T]
        xt4 = np.ascontiguousarray(
            xt.reshape(NB, P, NTB, P).transpose(2, 1, 0, 3))
        xt_list.append(xt4)

    # per-outfeature-shard weight-side tensors (shared by NT cores):
    # pack [q u8 | s bf16 | z bf16] per (b, ob) into one stream tensor
    pk_list, br_list = [], []
    for oc in range(NO):
        o0 = oc * OS
        qs = np.ascontiguousarray(qw[:, o0:o0 + OS])     # [256, OS] int64
        qbytes = qs.view(np.uint8).reshape(IN_F // 16, OS, 8)
        qb2 = np.ascontiguousarray(qbytes.transpose(0, 2, 1)).reshape(IN_F // 2, OS)
        qb = np.repeat(qb2, 2, axis=0)                   # [IN_F, OS]; row k
        qb_t = np.ascontiguousarray(
            qb.reshape(NB, P, NOB, OB).transpose(0, 2, 1, 3))

        s_bf = sc[:, o0:o0 + OS].astype(bf16).reshape(NB, NOB, OB)
        sr_t = np.ascontiguousarray(
            np.broadcast_to(s_bf[:, :, None, :], (NB, NOB, P, OB)))
        z_bf = zz[:, o0:o0 + OS].astype(bf16).reshape(NB, NOB, OB)
        zr_t = np.ascontiguousarray(
            np.broadcast_to(z_bf[:, :, None, :], (NB, NOB, P, OB)))

        blk = np.concatenate(
            [qb_t.astype(np.int16).view(np.uint8),
             sr_t.view(np.uint8), zr_t.view(np.uint8)],
            axis=-1)                                     # [NB, NOB, P, BLK]
        pk = np.ascontiguousarray(
            blk.reshape(NCH, CB, NOB, P, BLK)
               .transpose(0, 2, 3, 1, 4)
               .reshape(NCH, NOB, P, CB * BLK))
        pk_list.append(pk)
        br_list.append(np.ascontiguousarray(
            np.broadcast_to(bi[o0:o0 + OS], (P, OS))))

    in_maps = []
    for c in range(N_CORES):
        tc, oc = c // NO, c % NO
        in_maps.append({
            "xt": xt_list[tc],
            "pk": pk_list[oc],
            "br": br_list[oc],
            "sh": sh_np,
        })
    return in_maps


def get_program():
    if "nc" not in _CACHE:
        _CACHE["nc"] = _build_program()
    return _CACHE["nc"]


def kernel(x, qweight, scales, qzeros, g_idx, bias):
    nc = get_program()
    in_maps = _host_prep(x, qweight, scales, qzeros, bias)
    res = run_bass_kernel_spmd(nc, in_maps, core_ids=list(range(N_CORES)))
    y = np.empty((TOK, OUT_F), dtype=np.float32)
    for c in range(N_CORES):
        tc, oc = c // NO, c % NO
        yt = res.results[c]["y"]                         # [NTB, NOB, P, OB]
        y[tc * T:(tc + 1) * T, oc * OS:(oc + 1) * OS] = (
            yt.transpose(0, 2, 1, 3).reshape(T, OS))
    return y


# revision 18
# speedup vs baseline: 1.3717x; 1.0159x over previous
"""GPTQ int4 quantized linear (CaiQuantLinear) on 8 Trainium2 NeuronCores.

y = x @ dequant(qweight, scales, qzeros) + bias
  x: [8192, 4096] f32, qweight: [256, 4096] int64 (16x 4-bit packed along
  infeatures), scales: [32, 4096] f32, qzeros: [32, 256] int64 (packed along
  outfeatures), g_idx = arange(4096)//128, bias: [4096] f32 -> y: [8192, 4096] f32

Sharding: 4 token-shards x 2 outfeature-shards = 8 cores. Core c handles
tokens [2048*(c//2), +2048) and outfeatures [2048*(c%2), +2048).

Device kernel (per core): the packed weights are shipped as one byte per
4-bit-pair row (row k holds the byte containing nibble k, for every o), so
unpack is a single fused per-partition shift+mask tensor_scalar; dequant is
two tensor_tensor ops against k-replicated scale/zero rows; the matmul
accumulates 32 k-tiles of [128,128]x[128,256] bf16 into PSUM, and the
evacuation adds the bias. All replication/transposition is host-side layout
prep so every DMA is a contiguous load.
"""

import sys

if "/opt/trn_rl_repo" not in sys.path:
    sys.path.insert(0, "/opt/trn_rl_repo")

import numpy as np
import ml_dtypes

import concourse.bass as bass  # noqa: F401  (registers mybir types)
import concourse.mybir as mybir
import concourse.tile as tile
from concourse import bacc
from concourse.bass_utils import run_bass_kernel_spmd

BF16 = mybir.dt.bfloat16
F32 = mybir.dt.float32
U8 = mybir.dt.uint8

N_CORES = 8
NT, NO = 4, 2          # token shards x outfeature shards
TOK, IN_F, OUT_F = 8192, 4096, 4096
T = TOK // NT          # 2048 tokens per core
OS = OUT_F // NO       # 2048 outfeatures per core
P = 128
NB = IN_F // P         # 32 contraction super-tiles
OB = 256               # outfeature block (psum free dim)
NOB = OS // OB         # 8
NTB = T // P           # 16 token blocks

_CACHE = {}


CB = 4                  # super-tiles per packed stream DMA
NCH = NB // CB          # 8 chunks
I16 = mybir.dt.int16
BLK = 2 * OB + 4 * OB   # 1536 bytes per b: [q i16 | s bf16 | z bf16]


def _build_program():
    nc = bacc.Bacc("TRN2", target_bir_lowering=False, debug=False,
                   num_devices=N_CORES)
    xt_ap = nc.dram_tensor("xt", [NTB, P, NB, P], BF16, kind="ExternalInput").ap()
    pk_ap = nc.dram_tensor("pk", [NCH, NOB, P, CB * BLK], U8,
                           kind="ExternalInput").ap()
    br_ap = nc.dram_tensor("br", [P, OS], F32, kind="ExternalInput").ap()
    sh_ap = nc.dram_tensor("sh", [P, 1], I16, kind="ExternalInput").ap()
    y_ap = nc.dram_tensor("y", [NTB, NOB, P, OB], F32, kind="ExternalOutput").ap()

    with tile.TileContext(nc) as tc:
        with tc.tile_pool(name="resident", bufs=1) as rpool, \
             tc.tile_pool(name="wset", bufs=2) as wpool, \
             tc.tile_pool(name="qstream", bufs=3) as qpool, \
             tc.tile_pool(name="ostream", bufs=4) as opool, \
             tc.tile_pool(name="psum", bufs=4, space="PSUM") as ppool, \
             tc.tile_pool(name="jpsum", bufs=1, space="PSUM") as jpool:
            sh_sb = rpool.tile([P, 1], I16)
            nc.sync.dma_start(sh_sb[:], sh_ap[:])
            br_sb = rpool.tile([P, OS], F32)
            nc.gpsimd.dma_start(br_sb[:], br_ap[:])
            # zeros rhs for HAM-warmup matmuls during the load phase
            wz = rpool.tile([P, OB], BF16)
            nc.gpsimd.memset(wz[:], 0.0)
            jp = jpool.tile([P, OB], F32)
            xt_sb = rpool.tile([P, NTB, NB, P], BF16)

            def produce_wset(ob, warm):
                wset = wpool.tile([P, NB, OB], BF16, tag="wset")
                for ch in range(NCH):
                    pk_sb = qpool.tile([P, CB * BLK], U8, tag="pk")
                    # first weight set stripes across both HWDGE rings so it
                    # lands at full aggregate bandwidth during the ramp
                    eng = nc.scalar if (warm and ch % 2) else nc.sync
                    eng.dma_start(pk_sb[:], pk_ap[ch, ob])
                    if ob < 2:
                        # junk matmul on the arrived bytes: bridges PE idle
                        # windows during the load ramp so the HAM clock-gate
                        # stays at 8/8 (jp is never read)
                        nc.tensor.matmul(
                            jp[:], pk_sb[:, :2 * P].bitcast(BF16), wz[:],
                            start=True, stop=True)
                    for l in range(CB):
                        b = ch * CB + l
                        base = l * BLK
                        qt = pk_sb[:, base:base + 2 * OB].bitcast(I16)
                        st = pk_sb[:, base + 2 * OB:base + 4 * OB].bitcast(BF16)
                        zt = pk_sb[:, base + 4 * OB:base + 6 * OB].bitcast(BF16)
                        wu = qpool.tile([P, OB], I16, tag="wu")
                        nc.vector.tensor_scalar(
                            out=wu[:], in0=qt, scalar1=sh_sb[:], scalar2=15,
                            op0=mybir.AluOpType.logical_shift_right,
                            op1=mybir.AluOpType.bitwise_and)
                        nc.vector.tensor_tensor(
                            wset[:, b, :], wu[:], zt, mybir.AluOpType.subtract)
                        nc.vector.tensor_tensor(
                            wset[:, b, :], wset[:, b, :], st,
                            mybir.AluOpType.mult)
                    if warm:
                        # junk matmul on the freshly dequantized chunk keeps
                        # the PE HAM-warm while the first wset streams in
                        # (jp is never read; real groups start with start=True)
                        nc.tensor.matmul(jp[:], wset[:, ch * CB, :P], wz[:],
                                         start=True, stop=True)
                return wset

            # warm the PE immediately and keep it warm through the first
            # weight-set load: a serial chain of GpSimd memsets (~3us each)
            # paces junk matmuls across the otherwise PE-idle window
            for _ in range(2):
                nc.tensor.matmul(jp[:], wz[:, :P], wz[:], start=True, stop=True)
            wset = produce_wset(0, warm=True)

            for tb in range(NTB):
                eng = nc.scalar if tb % 2 else nc.sync
                eng.dma_start(xt_sb[:, tb], xt_ap[tb])
                nc.tensor.matmul(jp[:], xt_sb[:, tb, 0, :], wz[:],
                                 start=True, stop=True)

            for ob in range(NOB):
                if ob > 0:
                    wset = produce_wset(ob, warm=(ob == 1))
                for tb in range(NTB):
                    ps = ppool.tile([P, OB], F32, tag="ps")
                    for b in range(NB):
                        nc.tensor.matmul(
                            ps[:], xt_sb[:, tb, b, :], wset[:, b, :],
                            start=(b == 0), stop=(b == NB - 1))
                    ot = opool.tile([P, OB], F32, tag="ot")
                    nc.vector.tensor_tensor(
                        ot[:], ps[:], br_sb[:, ob * OB:(ob + 1) * OB],
                        mybir.AluOpType.add)
                    nc.gpsimd.dma_start(y_ap[tb, ob], ot[:])

    nc.compile()
    return nc


def _host_prep(x, qweight, scales, qzeros, bias):
    """Per-core input maps: pure layout prep (transpose / byte-split /
    row-replication), no arithmetic on the quantized weights."""
    bf16 = ml_dtypes.bfloat16
    x = np.asarray(x, dtype=np.float32)
    qw = np.asarray(qweight).astype(np.int64, copy=False)
    sc = np.asarray(scales, dtype=np.float32)
    qz = np.asarray(qzeros).astype(np.int64, copy=False)
    bi = np.asarray(bias, dtype=np.float32)

    # zeros: unpack along outfeatures, +1 (pack() stored z-1)
    shifts = (np.arange(16, dtype=np.uint64) * np.uint64(4))
    zz = ((qz.astype(np.uint64)[:, :, None] >> shifts[None, None, :])
          & np.uint64(15)).reshape(qz.shape[0], -1).astype(np.float32) + 1.0

    sh_np = (4 * (np.arange(P, dtype=np.int16) % 2)).reshape(P, 1)

    # per-token-shard xT (shared by the NO cores in a shard row),
    # laid out per token-block so the first matmul group's lhsT arrives fast:
    # [NTB, P(k-part), NB, P(t)]
    xt_list = []
    for tc in range(NT):
        xs = x[tc * T:(tc + 1) * T]                      # [T, IN_F]
        xt = np.ascontiguousarray(xs.T).astype(bf16)     # [IN_F, T]
        xt4 = np.ascontiguousarray(
            xt.reshape(NB, P, NTB, P).transpose(2, 1, 0, 3))
        xt_list.append(xt4)

    # per-outfeature-shard weight-side tensors (shared by NT cores):
    # pack [q u8 | s bf16 | z bf16] per (b, ob) into one stream tensor
    pk_list, br_list = [], []
    for oc in range(NO):
        o0 = oc * OS
        qs = np.ascontiguousarray(qw[:, o0:o0 + OS])     # [256, OS] int64
        qbytes = qs.view(np.uint8).reshape(IN_F // 16, OS, 8)
        qb2 = np.ascontiguousarray(qbytes.transpose(0, 2, 1)).reshape(IN_F // 2, OS)
        qb = np.repeat(qb2, 2, axis=0)                   # [IN_F, OS]; row k
        qb_t = np.ascontiguousarray(
            qb.reshape(NB, P, NOB, OB).transpose(0, 2, 1, 3))

        s_bf = sc[:, o0:o0 + OS].astype(bf16).reshape(NB, NOB, OB)
        sr_t = np.ascontiguousarray(
            np.broadcast_to(s_bf[:, :, None, :], (NB, NOB, P, OB)))
        z_bf = zz[:, o0:o0 + OS].astype(bf16).reshape(NB, NOB, OB)
        zr_t = np.ascontiguousarray(
            np.broadcast_to(z_bf[:, :, None, :], (NB, NOB, P, OB)))

        blk = np.concatenate(
            [qb_t.astype(np.int16).view(np.uint8),
             sr_t.view(np.uint8), zr_t.view(np.uint8)],
            axis=-1)                                     # [NB, NOB, P, BLK]
        pk = np.ascontiguousarray(
            blk.reshape(NCH, CB, NOB, P, BLK)
               .transpose(0, 2, 3, 1, 4)
               .reshape(NCH, NOB, P, CB * BLK))
        pk_list.append(pk)
        br_list.append(np.ascontiguousarray(
            np.broadcast_to(bi[o0:o0 + OS], (P, OS))))

    in_maps = []
    for c in range(N_CORES):
        tc, oc = c // NO, c % NO
        in_maps.append({
            "xt": xt_list[tc],
            "pk": pk_list[oc],
            "br": br_list[oc],
            "sh": sh_np,
        })
    return in_maps


def get_program():
    if "nc" not in _CACHE:
        _CACHE["nc"] = _build_program()
    return _CACHE["nc"]


def kernel(x, qweight, scales, qzeros, g_idx, bias):
    nc = get_program()
    in_maps = _host_prep(x, qweight, scales, qzeros, bias)
    res = run_bass_kernel_spmd(nc, in_maps, core_ids=list(range(N_CORES)))
    y = np.empty((TOK, OUT_F), dtype=np.float32)
    for c in range(N_CORES):
        tc, oc = c // NO, c % NO
        yt = res.results[c]["y"]                         # [NTB, NOB, P, OB]
        y[tc * T:(tc + 1) * T, oc * OS:(oc + 1) * OS] = (
            yt.transpose(0, 2, 1, 3).reshape(T, OS))
    return y


# revision 23
# speedup vs baseline: 1.3975x; 1.0188x over previous
"""GPTQ int4 quantized linear (CaiQuantLinear) on 8 Trainium2 NeuronCores.

y = x @ dequant(qweight, scales, qzeros) + bias
  x: [8192, 4096] f32, qweight: [256, 4096] int64 (16x 4-bit packed along
  infeatures), scales: [32, 4096] f32, qzeros: [32, 256] int64 (packed along
  outfeatures), g_idx = arange(4096)//128, bias: [4096] f32 -> y: [8192, 4096] f32

Sharding: 4 token-shards x 2 outfeature-shards = 8 cores. Core c handles
tokens [2048*(c//2), +2048) and outfeatures [2048*(c%2), +2048).

Device kernel (per core): the packed weights are shipped as one byte per
4-bit-pair row (row k holds the byte containing nibble k, for every o), so
unpack is a single fused per-partition shift+mask tensor_scalar; dequant is
two tensor_tensor ops against k-replicated scale/zero rows; the matmul
accumulates 32 k-tiles of [128,128]x[128,256] bf16 into PSUM, and the
evacuation adds the bias. All replication/transposition is host-side layout
prep so every DMA is a contiguous load.
"""

import sys

if "/opt/trn_rl_repo" not in sys.path:
    sys.path.insert(0, "/opt/trn_rl_repo")

import numpy as np
import ml_dtypes

import concourse.bass as bass  # noqa: F401  (registers mybir types)
import concourse.mybir as mybir
import concourse.tile as tile
from concourse import bacc
from concourse.bass_utils import run_bass_kernel_spmd

BF16 = mybir.dt.bfloat16
F32 = mybir.dt.float32
U8 = mybir.dt.uint8

N_CORES = 8
NT, NO = 4, 2          # token shards x outfeature shards
TOK, IN_F, OUT_F = 8192, 4096, 4096
T = TOK // NT          # 2048 tokens per core
OS = OUT_F // NO       # 2048 outfeatures per core
P = 128
NB = IN_F // P         # 32 contraction super-tiles
OB = 256               # outfeature block (psum free dim)
NOB = OS // OB         # 8
NTB = T // P           # 16 token blocks

_CACHE = {}


CB = 4                  # super-tiles per packed stream DMA
NCH = NB // CB          # 8 chunks
I16 = mybir.dt.int16
BLK = 2 * OB + 4 * OB   # 1536 bytes per b: [q i16 | s bf16 | z bf16]


def _build_program():
    nc = bacc.Bacc("TRN2", target_bir_lowering=False, debug=False,
                   num_devices=N_CORES)
    xt_ap = nc.dram_tensor("xt", [NTB, P, NB, P], BF16, kind="ExternalInput").ap()
    pk_ap = nc.dram_tensor("pk", [NCH, NOB, P, CB * BLK], U8,
                           kind="ExternalInput").ap()
    br_ap = nc.dram_tensor("br", [P, OS], F32, kind="ExternalInput").ap()
    sh_ap = nc.dram_tensor("sh", [P, 1], I16, kind="ExternalInput").ap()
    y_ap = nc.dram_tensor("y", [NTB, NOB, P, OB], F32, kind="ExternalOutput").ap()

    with tile.TileContext(nc) as tc:
        with tc.tile_pool(name="resident", bufs=1) as rpool, \
             tc.tile_pool(name="wset", bufs=2) as wpool, \
             tc.tile_pool(name="qstream", bufs=4) as qpool, \
             tc.tile_pool(name="ostream", bufs=6) as opool, \
             tc.tile_pool(name="psum", bufs=4, space="PSUM") as ppool, \
             tc.tile_pool(name="jpsum", bufs=1, space="PSUM") as jpool:
            sh_sb = rpool.tile([P, 1], I16)
            nc.sync.dma_start(sh_sb[:], sh_ap[:])
            br_sb = rpool.tile([P, OS], F32)
            nc.gpsimd.dma_start(br_sb[:], br_ap[:])
            # zeros rhs for HAM-warmup matmuls during the load phase
            wz = rpool.tile([P, OB], BF16)
            nc.gpsimd.memset(wz[:], 0.0)
            jp = jpool.tile([P, OB], F32)
            xt_sb = rpool.tile([P, NTB, NB, P], BF16)

            def produce_wset(ob, warm):
                wset = wpool.tile([P, NB, OB], BF16, tag="wset")
                for ch in range(NCH):
                    pk_sb = qpool.tile([P, CB * BLK], U8, tag="pk")
                    # first weight set stripes across both HWDGE rings so it
                    # lands at full aggregate bandwidth during the ramp
                    eng = nc.scalar if (warm and ch % 2) else nc.sync
                    if warm:
                        # half-chunk DMAs: dequant of the first super-tiles
                        # starts as soon as the first half lands
                        h = CB * BLK // 2
                        eng.dma_start(pk_sb[:, :h], pk_ap[ch, ob][:, :h])
                        eng.dma_start(pk_sb[:, h:], pk_ap[ch, ob][:, h:])
                    else:
                        eng.dma_start(pk_sb[:], pk_ap[ch, ob])
                    if ob < 2:
                        # junk matmul on the arrived bytes: bridges PE idle
                        # windows during the load ramp so the HAM clock-gate
                        # stays at 8/8 (jp is never read)
                        nc.tensor.matmul(
                            jp[:], pk_sb[:, :2 * P].bitcast(BF16), wz[:],
                            start=True, stop=True)
                    for l in range(CB):
                        b = ch * CB + l
                        base = l * BLK
                        qt = pk_sb[:, base:base + 2 * OB].bitcast(I16)
                        st = pk_sb[:, base + 2 * OB:base + 4 * OB].bitcast(BF16)
                        zt = pk_sb[:, base + 4 * OB:base + 6 * OB].bitcast(BF16)
                        wu = qpool.tile([P, OB], I16, tag="wu")
                        nc.vector.tensor_scalar(
                            out=wu[:], in0=qt, scalar1=sh_sb[:], scalar2=15,
                            op0=mybir.AluOpType.logical_shift_right,
                            op1=mybir.AluOpType.bitwise_and)
                        nc.vector.tensor_tensor(
                            wset[:, b, :], wu[:], zt, mybir.AluOpType.subtract)
                        nc.vector.tensor_tensor(
                            wset[:, b, :], wset[:, b, :], st,
                            mybir.AluOpType.mult)
                    if warm:
                        # junk matmul on the freshly dequantized chunk keeps
                        # the PE HAM-warm while the first wset streams in
                        # (jp is never read; real groups start with start=True)
                        nc.tensor.matmul(jp[:], wset[:, ch * CB, :P], wz[:],
                                         start=True, stop=True)
                return wset

            # warm the PE immediately and keep it warm through the first
            # weight-set load: a serial chain of GpSimd memsets (~3us each)
            # paces junk matmuls across the otherwise PE-idle window
            for _ in range(2):
                nc.tensor.matmul(jp[:], wz[:, :P], wz[:], start=True, stop=True)
            wset = produce_wset(0, warm=True)

            for tb in range(NTB):
                eng = nc.scalar if tb % 2 else nc.sync
                eng.dma_start(xt_sb[:, tb], xt_ap[tb])
                nc.tensor.matmul(jp[:], xt_sb[:, tb, 0, :], wz[:],
                                 start=True, stop=True)

            def evac(pslice, tb, ob):
                ot = opool.tile([P, OB], F32, tag="ot")
                nc.vector.tensor_tensor(
                    ot[:], pslice, br_sb[:, ob * OB:(ob + 1) * OB],
                    mybir.AluOpType.add)
                nc.gpsimd.dma_start(y_ap[tb, ob], ot[:])

            for ob in range(NOB):
                if ob > 0:
                    wset = produce_wset(ob, warm=(ob == 1))
                if ob == 0:
                    # wset[b] tiles stream in at dequant pace here; a b-outer
                    # emission over 4 concurrent accumulation groups lets the
                    # PE consume each weight tile the moment it's ready
                    # instead of head-of-line blocking on tb-group 0
                    for quarter in range(4):
                        pst = [ppool.tile([P, OB], F32, tag="ps",
                                          name=f"ps0_{quarter}_{i}")
                               for i in range(4)]
                        for b in range(NB):
                            for j in range(4):
                                nc.tensor.matmul(
                                    pst[j][:], xt_sb[:, quarter * 4 + j, b, :],
                                    wset[:, b, :],
                                    start=(b == 0), stop=(b == NB - 1))
                        for j in range(4):
                            evac(pst[j][:], quarter * 4 + j, ob)
                else:
                    for tb in range(NTB):
                        ps = ppool.tile([P, OB], F32, tag="ps")
                        for b in range(NB):
                            nc.tensor.matmul(
                                ps[:], xt_sb[:, tb, b, :], wset[:, b, :],
                                start=(b == 0), stop=(b == NB - 1))
                        evac(ps[:], tb, ob)

    nc.compile()
    return nc


def _host_prep(x, qweight, scales, qzeros, bias):
    """Per-core input maps: pure layout prep (transpose / byte-split /
    row-replication), no arithmetic on the quantized weights."""
    bf16 = ml_dtypes.bfloat16
    x = np.asarray(x, dtype=np.float32)
    qw = np.asarray(qweight).astype(np.int64, copy=False)
    sc = np.asarray(scales, dtype=np.float32)
    qz = np.asarray(qzeros).astype(np.int64, copy=False)
    bi = np.asarray(bias, dtype=np.float32)

    # zeros: unpack along outfeatures, +1 (pack() stored z-1)
    shifts = (np.arange(16, dtype=np.uint64) * np.uint64(4))
    zz = ((qz.astype(np.uint64)[:, :, None] >> shifts[None, None, :])
          & np.uint64(15)).reshape(qz.shape[0], -1).astype(np.float32) + 1.0

    sh_np = (4 * (np.arange(P, dtype=np.int16) % 2)).reshape(P, 1)

    # per-token-shard xT (shared by the NO cores in a shard row),
    # laid out per token-block so the first matmul group's lhsT arrives fast:
    # [NTB, P(k-part), NB, P(t)]
    xt_list = []
    for tc in range(NT):
        xs = x[tc * T:(tc + 1) * T]                      # [T, IN_F]
        xt = np.ascontiguousarray(xs.T).astype(bf16)     # [IN_F, T]
        xt4 = np.ascontiguousarray(
            xt.reshape(NB, P, NTB, P).transpose(2, 1, 0, 3))
        xt_list.append(xt4)

    # per-outfeature-shard weight-side tensors (shared by NT cores):
    # pack [q u8 | s bf16 | z bf16] per (b, ob) into one stream tensor
    pk_list, br_list = [], []
    for oc in range(NO):
        o0 = oc * OS
        qs = np.ascontiguousarray(qw[:, o0:o0 + OS])     # [256, OS] int64
        qbytes = qs.view(np.uint8).reshape(IN_F // 16, OS, 8)
        qb2 = np.ascontiguousarray(qbytes.transpose(0, 2, 1)).reshape(IN_F // 2, OS)
        qb = np.repeat(qb2, 2, axis=0)                   # [IN_F, OS]; row k
        qb_t = np.ascontiguousarray(
            qb.reshape(NB, P, NOB, OB).transpose(0, 2, 1, 3))

        s_bf = sc[:, o0:o0 + OS].astype(bf16).reshape(NB, NOB, OB)
        sr_t = np.ascontiguousarray(
            np.broadcast_to(s_bf[:, :, None, :], (NB, NOB, P, OB)))
        z_bf = zz[:, o0:o0 + OS].astype(bf16).reshape(NB, NOB, OB)
        zr_t = np.ascontiguousarray(
            np.broadcast_to(z_bf[:, :, None, :], (NB, NOB, P, OB)))

        blk = np.concatenate(
            [qb_t.astype(np.int16).view(np.uint8),
             sr_t.view(np.uint8), zr_t.view(np.uint8)],
            axis=-1)                                     # [NB, NOB, P, BLK]
        pk = np.ascontiguousarray(
            blk.reshape(NCH, CB, NOB, P, BLK)
               .transpose(0, 2, 3, 1, 4)
               .reshape(NCH, NOB, P, CB * BLK))
        pk_list.append(pk)
        br_list.append(np.ascontiguousarray(
            np.broadcast_to(bi[o0:o0 + OS], (P, OS))))

    in_maps = []
    for c in range(N_CORES):
        tc, oc = c // NO, c % NO
        in_maps.append({
            "xt": xt_list[tc],
            "pk": pk_list[oc],
            "br": br_list[oc],
            "sh": sh_np,
        })
    return in_maps


def get_program():
    if "nc" not in _CACHE:
        _CACHE["nc"] = _build_program()
    return _CACHE["nc"]


def kernel(x, qweight, scales, qzeros, g_idx, bias):
    nc = get_program()
    in_maps = _host_prep(x, qweight, scales, qzeros, bias)
    res = run_bass_kernel_spmd(nc, in_maps, core_ids=list(range(N_CORES)))
    y = np.empty((TOK, OUT_F), dtype=np.float32)
    for c in range(N_CORES):
        tc, oc = c // NO, c % NO
        yt = res.results[c]["y"]                         # [NTB, NOB, P, OB]
        y[tc * T:(tc + 1) * T, oc * OS:(oc + 1) * OS] = (
            yt.transpose(0, 2, 1, 3).reshape(T, OS))
    return y
